# revision 2
# baseline (speedup 1.0000x reference)
"""AttentiveMLP GNN message-passing kernel for 8 Trainium2 NeuronCores.

Sharding: edges are partitioned BY DESTINATION NODE (each core owns N/8 nodes
plus all their incoming edges) so no cross-core collectives are needed. Nodes
are grouped on the host into exact-degree classes (a pure layout/permutation
choice); within a class every node has exactly d edges, so segment softmax and
the attention-weighted aggregation are static dense reductions over [nodes, d]
tiles. Aggregation uses linearity: ctx = (sum alpha_e ef_e) @ W_et + b_et.

Device layout: class arrays are staged host-side in the exact SBUF layout
[128 partitions, T*d] (node row = t*128 + p), so each class is one contiguous
per-partition DMA. Edge/node features travel as bf16 to halve DMA traffic.
"""
import os
import numpy as np
import ml_dtypes
from contextlib import ExitStack

import concourse.bass as bass
import concourse.bacc as bacc
import concourse.tile as tile
import concourse.mybir as mybir
from concourse.bass_utils import run_bass_kernel_spmd

N_NODES = 100000
N_EDGES = 1600000
EF = 16
HID = 32
NF = 128
NCORES = 8
CHUNK = 512

f32 = mybir.dt.float32
bf16 = mybir.dt.bfloat16
BF = ml_dtypes.bfloat16


BUCKETS = [4, 6, 8, 10, 12, 14, 16, 18, 20, 22, 24, 28, 32, 64, 128,
           256, 512, 1024, 4096, 16384, 65536, 262144, 1048576, 2097152]


def _bucket_of(deg):
    b = np.zeros_like(deg)
    nz = deg > 0
    idx = np.searchsorted(np.asarray(BUCKETS), deg[nz])
    b[nz] = np.asarray(BUCKETS)[idx]
    return b


def _build_plan(dst):
    deg = np.bincount(dst, minlength=N_NODES)
    deg = _bucket_of(deg)
    order = np.argsort(deg, kind="stable")
    sdeg = deg[order]
    uniq, starts, counts = np.unique(sdeg, return_index=True, return_counts=True)
    ncls = len(uniq)
    rank = np.arange(N_NODES) - np.repeat(starts, counts)
    dev = rank % NCORES
    row_in_class = rank // NCORES
    n_pad = (counts + NCORES - 1) // NCORES
    n_pad = ((n_pad + 127) // 128) * 128   # 128-aligned class rows/offsets

    cls_ids = [ci for ci in range(ncls) if uniq[ci] > 0]
    cls_ids.sort(key=lambda ci: int(uniq[ci]) * int(n_pad[ci]))
    if uniq[0] == 0:
        cls_ids = cls_ids + [0]
    offs_arr = np.zeros(ncls, dtype=np.int64)
    acc = 0
    for ci in cls_ids:
        offs_arr[ci] = acc
        acc += n_pad[ci]
    R = int(acc)
    R_pad = ((R + CHUNK - 1) // CHUNK) * CHUNK
    R_mlp = ((R_pad + 4 * CHUNK - 1) // (4 * CHUNK)) * 4 * CHUNK

    cls_of_pos = np.repeat(np.arange(ncls), counts)
    lrow = offs_arr[cls_of_pos] + row_in_class

    node_dev = np.empty(N_NODES, dtype=np.int64)
    node_lrow = np.empty(N_NODES, dtype=np.int64)
    node_dev[order] = dev
    node_lrow[order] = lrow

    classes = [(int(uniq[ci]), int(n_pad[ci]), int(offs_arr[ci])) for ci in cls_ids]
    deg0_rows = classes[-1][1] if classes and classes[-1][0] == 0 else 0
    kclasses = [c for c in classes if c[0] > 0]
    zero_tail_start = R - deg0_rows

    return dict(
        uniq=uniq, counts=counts, node_dev=node_dev, node_lrow=node_lrow,
        R=R, R_pad=R_pad, R_mlp=R_mlp, kclasses=kclasses,
        zero_tail_start=zero_tail_start,
    )


def _mlpcol(r):
    # node row -> transposed-MLP column index (see aggT layout in the kernel)
    t = r // 128
    b = t // 16
    lt = t % 16
    q = (r % 128) // 32
    c = r % 32
    chunk = 4 * b + q
    return chunk * CHUNK + 32 * lt + c


def _shard_inputs(inputs, plan):
    lg = np.ascontiguousarray(
        np.asarray(inputs["edge_logits"], dtype=np.float32).reshape(-1))
    ef = np.ascontiguousarray(np.asarray(inputs["edge_feats"], dtype=np.float32))
    nf = np.asarray(inputs["node_feats"], dtype=np.float32)
    dst = np.asarray(inputs["dst"])
    W_et = np.asarray(inputs["W_et"], dtype=np.float32)
    b_et = np.asarray(inputs["b_et"], dtype=np.float32)
    W1 = np.asarray(inputs["W1"], dtype=np.float32)
    b1 = np.asarray(inputs["b1"], dtype=np.float32)
    W2 = np.asarray(inputs["W2"], dtype=np.float32)
    b2 = np.asarray(inputs["b2"], dtype=np.float32)

    node_dev, node_lrow = plan["node_dev"], plan["node_lrow"]
    R, R_pad, R_mlp = plan["R"], plan["R_pad"], plan["R_mlp"]
    kclasses = plan["kclasses"]
    uniq, counts = plan["uniq"], plan["counts"]

    ekey = node_dev[dst] * R + node_lrow[dst]
    eorder = np.argsort(ekey, kind="stable")
    # slot index of each (sorted) edge within its node
    sk = ekey[eorder]
    newrun = np.empty(N_EDGES, dtype=bool)
    newrun[0] = True
    newrun[1:] = sk[1:] != sk[:-1]
    runstart = np.maximum.accumulate(np.where(newrun, np.arange(N_EDGES), 0))
    slot = np.arange(N_EDGES) - runstart
    e_dev = node_dev[dst[eorder]]
    e_lrow = node_lrow[dst[eorder]]
    lg_s = lg[eorder]
    ef_s = ef[eorder].astype(BF)

    in_maps = [dict() for _ in range(NCORES)]
    for dv in range(NCORES):
        dmask = e_dev == dv
        d_lrow = e_lrow[dmask]
        d_slot = slot[dmask]
        d_lg = lg_s[dmask]
        d_ef = ef_s[dmask]
        for idx, (d, npad, off) in enumerate(kclasses):
            T = (npad + 127) // 128
            cmask = (d_lrow >= off) & (d_lrow < off + npad)
            r = d_lrow[cmask] - off
            s = d_slot[cmask]
            p = r % 128
            t = r // 128
            # host layout [128, T*d]: row r=(t*128+p) -> [p, t*d + s]
            flat_lg = np.full((128, T * d), -1.0e30, dtype=BF)
            flat_lg[p, t * d + s] = d_lg[cmask]
            # all-pad rows: lg=0 so den=d (finite) instead of 0 -> xs stays
            # finite (prod is still 0 because ef is 0); avoids NaN leaking
            # through unwritten a16b channels into ctx4
            real = np.zeros(T * 128, dtype=bool)
            real[r] = True
            er = np.nonzero(~real)[0]
            if er.size:
                cols = (er // 128)[:, None] * d + np.arange(d)[None, :]
                flat_lg[er % 128][:, :] = flat_lg[er % 128]  # noop guard
                flat_lg[(er % 128)[:, None], cols] = 0.0
            # feature-major slots: [p, t*d*EF + f*d + s]
            flat_ef = np.zeros((128, T * d * EF), dtype=BF)
            col = (t * d * EF + s)[:, None] + np.arange(EF)[None, :] * d
            flat_ef[p[:, None], col] = d_ef[cmask]
            in_maps[dv][f"lg{idx}"] = flat_lg
            in_maps[dv][f"ef{idx}"] = flat_ef

    for dv in range(NCORES):
        sel = node_dev == dv
        nid = np.nonzero(sel)[0]
        lr = node_lrow[sel]
        nf_dev = np.zeros((R_mlp, NF), dtype=np.float32)
        nf_dev[_mlpcol(lr)] = nf[nid]
        in_maps[dv]["nfT"] = np.ascontiguousarray(nf_dev.T).astype(BF)

    wet4 = np.zeros((128, 128), dtype=BF)
    bet4 = np.zeros((128, 1), dtype=np.float32)
    for g in range(4):
        wet4[32 * g:32 * g + EF, 32 * g:32 * g + HID] = W_et.astype(BF)
        bet4[32 * g:32 * g + HID, 0] = b_et
    consts = {
        "wet4": wet4,
        "bet4": bet4,
        "w1c": np.ascontiguousarray(np.tile(W1[:HID], (4, 1))).astype(BF),
        "w1n": np.ascontiguousarray(W1[HID:]).astype(BF),
        "b1": (b1 - W1[:HID].sum(axis=0)).reshape(NF, 1).astype(np.float32),
        "w2": W2.astype(BF),
        "b2": b2.reshape(NF, 1).astype(np.float32),
    }
    for dv in range(NCORES):
        in_maps[dv].update({k: v.copy() for k, v in consts.items()})
    return in_maps


def _unshard(results, plan):
    node_dev, node_lrow = plan["node_dev"], plan["node_lrow"]
    out = np.empty((N_NODES, NF), dtype=np.float32)
    for dv in range(NCORES):
        sel = node_dev == dv
        nid = np.nonzero(sel)[0]
        lr = node_lrow[sel]
        out_dev = results[dv]["outT"].T.astype(np.float32)
        out[nid] = out_dev[_mlpcol(lr)]
    return out


def _build_kernel(plan):
    kclasses = plan["kclasses"]
    R_pad = plan["R_pad"]
    R_mlp = plan["R_mlp"]
    zts = plan["zero_tail_start"]
    n_blocks = R_mlp // (4 * CHUNK)
    n_chunks = 4 * n_blocks

    nc = bacc.Bacc("TRN2", target_bir_lowering=False, debug=False,
                   num_devices=NCORES)

    lg_d, ef_d = [], []
    for idx, (d, npad, off) in enumerate(kclasses):
        T = (npad + 127) // 128
        lg_d.append(nc.dram_tensor(f"lg{idx}", [128, T * d], bf16, kind="ExternalInput"))
        ef_d.append(nc.dram_tensor(f"ef{idx}", [128, T * d * EF], bf16, kind="ExternalInput"))
    nfT_d = nc.dram_tensor("nfT", [NF, R_mlp], bf16, kind="ExternalInput")
    wet4_d = nc.dram_tensor("wet4", [128, 128], bf16, kind="ExternalInput")
    bet4_d = nc.dram_tensor("bet4", [128, 1], f32, kind="ExternalInput")
    w1c_d = nc.dram_tensor("w1c", [NF, NF], bf16, kind="ExternalInput")
    w1n_d = nc.dram_tensor("w1n", [NF, NF], bf16, kind="ExternalInput")
    b1_d = nc.dram_tensor("b1", [NF, 1], f32, kind="ExternalInput")
    w2_d = nc.dram_tensor("w2", [NF, NF], bf16, kind="ExternalInput")
    b2_d = nc.dram_tensor("b2", [NF, 1], f32, kind="ExternalInput")
    out_d = nc.dram_tensor("outT", [NF, R_mlp], bf16, kind="ExternalOutput")

    with tile.TileContext(nc) as tc, ExitStack() as ctx:
        const_pool = ctx.enter_context(tc.tile_pool(name="const", bufs=1))
        agg_pool = ctx.enter_context(tc.tile_pool(name="agg", bufs=1))
        cls_pool = ctx.enter_context(tc.tile_pool(name="cls", bufs=4))
        work_pool = ctx.enter_context(tc.tile_pool(name="work", bufs=3))
        small_pool = ctx.enter_context(tc.tile_pool(name="small", bufs=4))
        mlp_pool = ctx.enter_context(tc.tile_pool(name="mlp", bufs=3))
        ctx_pool = ctx.enter_context(tc.tile_pool(name="ctxs", bufs=2))
        ctx_psum = ctx.enter_context(tc.tile_pool(name="ctxp", bufs=2, space="PSUM"))
        mlp1_psum = ctx.enter_context(tc.tile_pool(name="m1p", bufs=3, space="PSUM"))
        mlp2_psum = ctx.enter_context(tc.tile_pool(name="m2p", bufs=3, space="PSUM"))

        # prefetch first class's edge data before the const loads so phase A
        # starts immediately
        prefetched = {}
        for pi in range(min(2, len(kclasses))):
            dp, npadp, _ = kclasses[pi]
            Tp = (npadp + 127) // 128
            lgtp = cls_pool.tile([128, Tp * dp], bf16, tag="lg", name=f"lgt_pre{pi}")
            eftp = cls_pool.tile([128, Tp * dp * EF], bf16, tag="ef", name=f"eft_pre{pi}")
            nc.scalar.dma_start(lgtp[:], lg_d[pi].ap())
            nc.sync.dma_start(eftp[:], ef_d[pi].ap())
            prefetched[pi] = (lgtp, eftp)

        def load_const(name, dram, shape, dtype=f32):
            t = const_pool.tile(shape, dtype, name=name)
            nc.gpsimd.dma_start(t[:], dram.ap())
            return t

        wet4 = load_const("wet4c", wet4_d, [128, 128], bf16)
        bet4 = load_const("bet4c", bet4_d, [128, 1])
        w1c = load_const("w1cc", w1c_d, [NF, NF], bf16)
        w1n = load_const("w1nc", w1n_d, [NF, NF], bf16)
        w2 = load_const("w2c", w2_d, [NF, NF], bf16)
        b1 = load_const("b1c", b1_d, [NF, 1])
        b2 = load_const("b2c", b2_d, [NF, 1])
        zeros = const_pool.tile([128, CHUNK], f32, name="zeros")
        nc.gpsimd.memset(zeros[:], 0.0)



        aggT_blocks = []
        n_real_tiles = plan["R"] // 128
        for b in range(n_blocks):
            ab = agg_pool.tile([128, CHUNK], bf16, name=f"aggT{b}")
            used = max(0, min(16, n_real_tiles - 16 * b)) * 32
            if used < CHUNK:
                nc.gpsimd.memset(ab[0:128, used:CHUNK], 0.0)
            aggT_blocks.append(ab)

        # deg0 rows -> (block, q, col ranges) in the transposed-MLP space
        zero_ranges = {}
        if zts < plan["R"]:
            rows = np.arange(zts, plan["R"])
            cols = _mlpcol(rows)
            ch = cols // CHUNK
            cc = cols % CHUNK
            for b in range(n_blocks):
                for q in range(4):
                    sel = ch == 4 * b + q
                    if not sel.any():
                        continue
                    cs = np.sort(cc[sel])
                    runs = []
                    r0 = prev = cs[0]
                    for v in cs[1:]:
                        if v != prev + 1:
                            runs.append((q, int(r0), int(prev) + 1))
                            r0 = v
                        prev = v
                    runs.append((q, int(r0), int(prev) + 1))
                    zero_ranges.setdefault(b, []).extend(runs)

        # ---------------- Phases B+C ----------------
        def emit_block(b):
            ctx4 = ctx_psum.tile([128, CHUNK], f32, tag="ctx4", name=f"ctx4_{b}")
            nc.tensor.matmul(ctx4[:], wet4[:], aggT_blocks[b][:])
            s1 = mlp_pool.tile([128, CHUNK], f32, tag="s1", name=f"s1_{b}")
            nc.scalar.activation(s1[:], ctx4[:], mybir.ActivationFunctionType.Relu,
                                 bias=bet4[:], scale=1.0)
            s2 = mlp_pool.tile([128, CHUNK], f32, tag="s2", name=f"s2_{b}")
            nc.scalar.activation(s2[:], ctx4[:], mybir.ActivationFunctionType.Exp,
                                 bias=bet4[:], scale=1.0)
            cb = ctx_pool.tile([128, CHUNK], bf16, tag="cb", name=f"cb_{b}")
            nc.vector.scalar_tensor_tensor(cb[:], s2[:], 1.0, s1[:],
                                           mybir.AluOpType.min,
                                           mybir.AluOpType.add)
            # zero the context of deg-0 nodes (rows in [zts, R))
            for (q, c0, c1) in zero_ranges.get(b, []):
                nc.vector.memset(cb[32 * q:32 * (q + 1), c0:c1], 0.0)

            bcols = 4 * CHUNK
            nfblk = mlp_pool.tile([NF, 4 * CHUNK], bf16, tag="nfblk", name=f"nfblk{b}")
            nc.sync.dma_start(nfblk[:], nfT_d.ap()[:, b * 4 * CHUNK:(b + 1) * 4 * CHUNK])
            oblk = mlp_pool.tile([NF, 4 * CHUNK], bf16, tag="oblk", name=f"oblk{b}")
            for g in range(4):
                j = 4 * b + g
                if j >= n_chunks:
                    break
                nfb = nfblk[0:NF, g * CHUNK:(g + 1) * CHUNK]
                ps1 = mlp1_psum.tile([NF, CHUNK], f32, tag="ps1", name=f"ps1_{j}")
                if g < 3:
                    cbg = cb[32 * g:32 * (g + 1), :]
                    w1cg = w1c[32 * g:32 * (g + 1), :]
                else:
                    cb3 = mlp_pool.tile([HID, CHUNK], bf16, tag="cb3", name=f"cb3_{j}")
                    nc.vector.tensor_copy(cb3[:], cb[96:128, :])
                    cbg = cb3[:]
                    w1cg = w1c[0:HID, :]
                nc.tensor.matmul(ps1[:], w1cg, cbg, start=True, stop=False)
                nc.tensor.matmul(ps1[:], w1n[:], nfb, start=False, stop=True)
                h = mlp_pool.tile([NF, CHUNK], bf16, tag="h", name=f"h{j}")
                late = b >= (n_blocks + 1) // 2
                if late:
                    nc.vector.scalar_tensor_tensor(h[:], ps1[:], b1[:], zeros[:],
                                                   mybir.AluOpType.add,
                                                   mybir.AluOpType.max)
                else:
                    nc.scalar.activation(h[:], ps1[:],
                                         mybir.ActivationFunctionType.Relu,
                                         bias=b1[:], scale=1.0)
                ps2 = mlp2_psum.tile([NF, CHUNK], f32, tag="ps2", name=f"ps2_{j}")
                nc.tensor.matmul(ps2[:], w2[:], h[:])
                ov = oblk[0:NF, g * CHUNK:(g + 1) * CHUNK]
                if late:
                    nc.vector.scalar_tensor_tensor(ov, ps2[:], b2[:], zeros[:],
                                                   mybir.AluOpType.add,
                                                   mybir.AluOpType.max)
                else:
                    nc.scalar.activation(ov, ps2[:],
                                         mybir.ActivationFunctionType.Relu,
                                         bias=b2[:], scale=1.0)
            nc.gpsimd.dma_start(out_d.ap()[:, b * 4 * CHUNK:b * 4 * CHUNK + bcols],
                                oblk[0:NF, 0:bcols])


        next_block = [0]

        # ---------------- Phase A ----------------
        for idx, (d, npad, off) in enumerate(kclasses):
            T = (npad + 127) // 128
            if idx in prefetched:
                lgt, eft = prefetched[idx]
            else:
                lgt = cls_pool.tile([128, T * d], bf16, tag="lg", name=f"lgt{idx}")
                eft = cls_pool.tile([128, T * d * EF], bf16, tag="ef", name=f"eft{idx}")
                nc.scalar.dma_start(lgt[:], lg_d[idx].ap())
                nc.sync.dma_start(eft[:], ef_d[idx].ap())

            x = work_pool.tile([128, T * d], bf16, tag="x", name=f"x{idx}")
            nc.scalar.activation(x[:], lgt[:], mybir.ActivationFunctionType.Exp)
            x3 = x[:].rearrange("p (t d) -> p t d", t=T)
            den = small_pool.tile([128, T], f32, tag="den", name=f"den{idx}")
            nc.vector.tensor_reduce(den[:], x3, mybir.AxisListType.X,
                                    mybir.AluOpType.add)
            rd = small_pool.tile([128, T], f32, tag="rd", name=f"rd{idx}")
            nc.vector.reciprocal(rd[:], den[:])
            xs = work_pool.tile([128, T * d], bf16, tag="xs", name=f"xs{idx}")
            xs3 = xs[:].rearrange("p (t d) -> p t d", t=T)
            rd_b = rd[:].unsqueeze(2).broadcast_to([128, T, d])
            nc.vector.tensor_tensor(xs3, x3, rd_b, mybir.AluOpType.mult)

            prod = work_pool.tile([128, T * d * EF], bf16, tag="prod", name=f"prod{idx}")
            ef4 = eft[:].rearrange("p (t f d) -> p t f d", t=T, f=EF)
            xs4 = xs3.unsqueeze(2).broadcast_to([128, T, EF, d])
            prod4 = prod[:].rearrange("p (t f d) -> p t f d", t=T, f=EF)
            nc.vector.tensor_tensor(prod4, ef4, xs4, mybir.AluOpType.mult)

            # pairwise halving (contiguous halves keep 2x bf16 mode) before
            # the 1x tensor_reduce: cuts the dominant reduce cost ~40%
            cur, w = prod, d
            lvl = 0
            while w % 2 == 0 and w > 2 and lvl < 2:
                h = w // 2
                nxt = work_pool.tile([128, T * EF * h], bf16, tag=f"ph{lvl}",
                                     name=f"ph{lvl}_{idx}")
                pa = bass.AP(cur.tensor, 0,
                             [[T * EF * w, 128], [w, T * EF], [1, h]])
                pb = bass.AP(cur.tensor, h,
                             [[T * EF * w, 128], [w, T * EF], [1, h]])
                po = bass.AP(nxt.tensor, 0,
                             [[T * EF * h, 128], [h, T * EF], [1, h]])
                nc.vector.tensor_tensor(po, pa, pb, mybir.AluOpType.add)
                cur, w = nxt, h
                lvl += 1
            a16 = work_pool.tile([128, T * EF], f32, tag="a16", name=f"a16{idx}")
            a16_3 = a16[:].rearrange("p (t f) -> p t f", t=T)
            prod_r = cur[:].rearrange("p (t f d) -> p t f d", t=T, f=EF, d=w)
            nc.vector.tensor_reduce(a16_3, prod_r, mybir.AxisListType.X,
                                    mybir.AluOpType.add)
            # padded bf16 cast: [128, T*16] -> channels 0:16 of [128, T*32]
            a16b = work_pool.tile([128, T * 32], bf16, tag="a16b", name=f"a16b{idx}")
            nc.scalar.copy(
                bass.AP(a16b.tensor, 0, [[T * 32, 128], [32, T], [EF, 2], [1, EF]]),
                bass.AP(a16.tensor, 0, [[T * EF, 128], [EF, T], [0, 2], [1, EF]]))
            # 32x32 block transpose straight into the aggT blocks
            t0 = off // 128
            t = 0
            while t < T:
                b = (t0 + t) // 16
                te = min(T, (b + 1) * 16 - t0)
                nc.vector.transpose(
                    aggT_blocks[b][0:128, 32 * (t0 + t - 16 * b):32 * (t0 + te - 16 * b)],
                    a16b[:, 32 * t:32 * te])
                t = te

            done_rows = off + npad
            if idx == len(kclasses) - 1:
                done_rows = R_mlp * 2
            while (next_block[0] < n_blocks
                   and done_rows >= min((next_block[0] + 1) * 4 * CHUNK, R_pad)):
                emit_block(next_block[0])
                next_block[0] += 1

    nc.compile()
    return nc


def kernel(**inputs):
    dst = np.asarray(inputs["dst"])
    plan = _build_plan(dst)
    in_maps = _shard_inputs(inputs, plan)
    nc = _build_kernel(plan)
    trace = bool(int(os.environ.get("GNN_PROFILE", "0")))
    if trace:
        try:
            _install_ntff_hook()
        except Exception:
            pass
    res = run_bass_kernel_spmd(nc, in_maps, core_ids=list(range(NCORES)),
                               trace=trace)
    kernel.last_results = res
    return _unshard(res.results, plan)


def _install_ntff_hook():
    """Recreate antenv.axon_hooks (absent in this image) so
    run_bass_kernel_spmd(trace=True) can NTFF-profile via libaxon_pjrt.so."""
    import contextlib, ctypes, sys, types
    if 'antenv.axon_hooks' in sys.modules:
        return
    lib = ctypes.CDLL('/opt/axon/libaxon_pjrt.so')
    lib.axon_start_nrt_profile.argtypes = [ctypes.POINTER(ctypes.c_int64), ctypes.c_size_t]
    lib.axon_start_nrt_profile.restype = ctypes.c_int64
    lib.axon_stop_nrt_profile.argtypes = [ctypes.c_char_p]
    lib.axon_stop_nrt_profile.restype = ctypes.c_int64

    @contextlib.contextmanager
    def _hook(output_dir, device_ids):
        import jax
        jax.devices()
        if device_ids:
            ids = (ctypes.c_int64 * len(device_ids))(*device_ids)
            rc = lib.axon_start_nrt_profile(ids, len(device_ids))
        else:
            rc = lib.axon_start_nrt_profile(None, 0)
        if rc != 0:
            raise RuntimeError(f"axon_start_nrt_profile rc={rc}")
        try:
            yield
        finally:
            n = lib.axon_stop_nrt_profile(str(output_dir).encode())
            print(f"ntff profile: {n} file(s) written to {output_dir}", file=sys.stderr)

    mod = types.ModuleType('antenv.axon_hooks')
    mod.get_axon_ntff_profile_hook = lambda: _hook
    mod.set_axon_ntff_profile_hook = lambda h: None
    import antenv
    antenv.axon_hooks = mod
    sys.modules['antenv.axon_hooks'] = mod



# revision 3
# speedup vs baseline: 1.0040x; 1.0040x over previous
"""AttentiveMLP GNN message-passing kernel for 8 Trainium2 NeuronCores.

Sharding: edges are partitioned BY DESTINATION NODE (each core owns N/8 nodes
plus all their incoming edges) so no cross-core collectives are needed. Nodes
are grouped on the host into exact-degree classes (a pure layout/permutation
choice); within a class every node has exactly d edges, so segment softmax and
the attention-weighted aggregation are static dense reductions over [nodes, d]
tiles. Aggregation uses linearity: ctx = (sum alpha_e ef_e) @ W_et + b_et.

Device layout: class arrays are staged host-side in the exact SBUF layout
[128 partitions, T*d] (node row = t*128 + p), so each class is one contiguous
per-partition DMA. Edge/node features travel as bf16 to halve DMA traffic.
"""
import os
import numpy as np
import ml_dtypes
from contextlib import ExitStack

import concourse.bass as bass
import concourse.bacc as bacc
import concourse.tile as tile
import concourse.mybir as mybir
from concourse.bass_utils import run_bass_kernel_spmd

N_NODES = 100000
N_EDGES = 1600000
EF = 16
HID = 32
NF = 128
NCORES = 8
CHUNK = 512

f32 = mybir.dt.float32
bf16 = mybir.dt.bfloat16
BF = ml_dtypes.bfloat16


BUCKETS = [4, 6, 8, 10, 12, 14, 16, 18, 20, 22, 24, 28, 32, 64, 128,
           256, 512, 1024, 4096, 16384, 65536, 262144, 1048576, 2097152]


def _bucket_of(deg):
    b = np.zeros_like(deg)
    nz = deg > 0
    idx = np.searchsorted(np.asarray(BUCKETS), deg[nz])
    b[nz] = np.asarray(BUCKETS)[idx]
    return b


def _build_plan(dst):
    deg = np.bincount(dst, minlength=N_NODES)
    deg = _bucket_of(deg)
    order = np.argsort(deg, kind="stable")
    sdeg = deg[order]
    uniq, starts, counts = np.unique(sdeg, return_index=True, return_counts=True)
    ncls = len(uniq)
    rank = np.arange(N_NODES) - np.repeat(starts, counts)
    dev = rank % NCORES
    row_in_class = rank // NCORES
    n_pad = (counts + NCORES - 1) // NCORES
    n_pad = ((n_pad + 127) // 128) * 128   # 128-aligned class rows/offsets

    cls_ids = [ci for ci in range(ncls) if uniq[ci] > 0]
    cls_ids.sort(key=lambda ci: int(uniq[ci]) * int(n_pad[ci]))
    if uniq[0] == 0:
        cls_ids = cls_ids + [0]
    offs_arr = np.zeros(ncls, dtype=np.int64)
    acc = 0
    for ci in cls_ids:
        offs_arr[ci] = acc
        acc += n_pad[ci]
    R = int(acc)
    R_pad = ((R + CHUNK - 1) // CHUNK) * CHUNK
    R_mlp = ((R_pad + 4 * CHUNK - 1) // (4 * CHUNK)) * 4 * CHUNK

    cls_of_pos = np.repeat(np.arange(ncls), counts)
    lrow = offs_arr[cls_of_pos] + row_in_class

    node_dev = np.empty(N_NODES, dtype=np.int64)
    node_lrow = np.empty(N_NODES, dtype=np.int64)
    node_dev[order] = dev
    node_lrow[order] = lrow

    classes = [(int(uniq[ci]), int(n_pad[ci]), int(offs_arr[ci])) for ci in cls_ids]
    deg0_rows = classes[-1][1] if classes and classes[-1][0] == 0 else 0
    kclasses = [c for c in classes if c[0] > 0]
    zero_tail_start = R - deg0_rows

    return dict(
        uniq=uniq, counts=counts, node_dev=node_dev, node_lrow=node_lrow,
        R=R, R_pad=R_pad, R_mlp=R_mlp, kclasses=kclasses,
        zero_tail_start=zero_tail_start,
    )


def _mlpcol(r):
    # node row -> transposed-MLP column index (see aggT layout in the kernel)
    t = r // 128
    b = t // 16
    lt = t % 16
    q = (r % 128) // 32
    c = r % 32
    chunk = 4 * b + q
    return chunk * CHUNK + 32 * lt + c


def _shard_inputs(inputs, plan):
    lg = np.ascontiguousarray(
        np.asarray(inputs["edge_logits"], dtype=np.float32).reshape(-1))
    ef = np.ascontiguousarray(np.asarray(inputs["edge_feats"], dtype=np.float32))
    nf = np.asarray(inputs["node_feats"], dtype=np.float32)
    dst = np.asarray(inputs["dst"])
    W_et = np.asarray(inputs["W_et"], dtype=np.float32)
    b_et = np.asarray(inputs["b_et"], dtype=np.float32)
    W1 = np.asarray(inputs["W1"], dtype=np.float32)
    b1 = np.asarray(inputs["b1"], dtype=np.float32)
    W2 = np.asarray(inputs["W2"], dtype=np.float32)
    b2 = np.asarray(inputs["b2"], dtype=np.float32)

    node_dev, node_lrow = plan["node_dev"], plan["node_lrow"]
    R, R_pad, R_mlp = plan["R"], plan["R_pad"], plan["R_mlp"]
    kclasses = plan["kclasses"]
    uniq, counts = plan["uniq"], plan["counts"]

    ekey = node_dev[dst] * R + node_lrow[dst]
    eorder = np.argsort(ekey, kind="stable")
    # slot index of each (sorted) edge within its node
    sk = ekey[eorder]
    newrun = np.empty(N_EDGES, dtype=bool)
    newrun[0] = True
    newrun[1:] = sk[1:] != sk[:-1]
    runstart = np.maximum.accumulate(np.where(newrun, np.arange(N_EDGES), 0))
    slot = np.arange(N_EDGES) - runstart
    e_dev = node_dev[dst[eorder]]
    e_lrow = node_lrow[dst[eorder]]
    lg_s = lg[eorder]
    ef_s = ef[eorder].astype(BF)

    in_maps = [dict() for _ in range(NCORES)]
    for dv in range(NCORES):
        dmask = e_dev == dv
        d_lrow = e_lrow[dmask]
        d_slot = slot[dmask]
        d_lg = lg_s[dmask]
        d_ef = ef_s[dmask]
        for idx, (d, npad, off) in enumerate(kclasses):
            T = (npad + 127) // 128
            cmask = (d_lrow >= off) & (d_lrow < off + npad)
            r = d_lrow[cmask] - off
            s = d_slot[cmask]
            p = r % 128
            t = r // 128
            # host layout [128, T*d]: row r=(t*128+p) -> [p, t*d + s]
            flat_lg = np.full((128, T * d), -1.0e30, dtype=BF)
            flat_lg[p, t * d + s] = d_lg[cmask]
            # all-pad rows: lg=0 so den=d (finite) instead of 0 -> xs stays
            # finite (prod is still 0 because ef is 0); avoids NaN leaking
            # through unwritten a16b channels into ctx4
            real = np.zeros(T * 128, dtype=bool)
            real[r] = True
            er = np.nonzero(~real)[0]
            if er.size:
                cols = (er // 128)[:, None] * d + np.arange(d)[None, :]
                flat_lg[er % 128][:, :] = flat_lg[er % 128]  # noop guard
                flat_lg[(er % 128)[:, None], cols] = 0.0
            # feature-major slots: [p, t*d*EF + f*d + s]
            flat_ef = np.zeros((128, T * d * EF), dtype=BF)
            col = (t * d * EF + s)[:, None] + np.arange(EF)[None, :] * d
            flat_ef[p[:, None], col] = d_ef[cmask]
            in_maps[dv][f"lg{idx}"] = flat_lg
            in_maps[dv][f"ef{idx}"] = flat_ef

    for dv in range(NCORES):
        sel = node_dev == dv
        nid = np.nonzero(sel)[0]
        lr = node_lrow[sel]
        nf_dev = np.zeros((R_mlp, NF), dtype=np.float32)
        nf_dev[_mlpcol(lr)] = nf[nid]
        in_maps[dv]["nfT"] = np.ascontiguousarray(nf_dev.T).astype(BF)

    wet4 = np.zeros((128, 128), dtype=BF)
    bet4 = np.zeros((128, 1), dtype=np.float32)
    for g in range(4):
        wet4[32 * g:32 * g + EF, 32 * g:32 * g + HID] = W_et.astype(BF)
        bet4[32 * g:32 * g + HID, 0] = b_et
    consts = {
        "wet4": wet4,
        "bet4": bet4,
        "w1c": np.ascontiguousarray(np.tile(W1[:HID], (4, 1))).astype(BF),
        "w1n": np.ascontiguousarray(W1[HID:]).astype(BF),
        "b1": (b1 - W1[:HID].sum(axis=0)).reshape(NF, 1).astype(np.float32),
        "w2": W2.astype(BF),
        "b2": b2.reshape(NF, 1).astype(np.float32),
    }
    for dv in range(NCORES):
        in_maps[dv].update({k: v.copy() for k, v in consts.items()})
    return in_maps


def _unshard(results, plan):
    node_dev, node_lrow = plan["node_dev"], plan["node_lrow"]
    out = np.empty((N_NODES, NF), dtype=np.float32)
    for dv in range(NCORES):
        sel = node_dev == dv
        nid = np.nonzero(sel)[0]
        lr = node_lrow[sel]
        out_dev = results[dv]["outT"].T.astype(np.float32)
        out[nid] = out_dev[_mlpcol(lr)]
    return out


def _build_kernel(plan):
    kclasses = plan["kclasses"]
    R_pad = plan["R_pad"]
    R_mlp = plan["R_mlp"]
    zts = plan["zero_tail_start"]
    n_blocks = R_mlp // (4 * CHUNK)
    n_chunks = 4 * n_blocks

    nc = bacc.Bacc("TRN2", target_bir_lowering=False, debug=False,
                   num_devices=NCORES)

    lg_d, ef_d = [], []
    for idx, (d, npad, off) in enumerate(kclasses):
        T = (npad + 127) // 128
        lg_d.append(nc.dram_tensor(f"lg{idx}", [128, T * d], bf16, kind="ExternalInput"))
        ef_d.append(nc.dram_tensor(f"ef{idx}", [128, T * d * EF], bf16, kind="ExternalInput"))
    nfT_d = nc.dram_tensor("nfT", [NF, R_mlp], bf16, kind="ExternalInput")
    wet4_d = nc.dram_tensor("wet4", [128, 128], bf16, kind="ExternalInput")
    bet4_d = nc.dram_tensor("bet4", [128, 1], f32, kind="ExternalInput")
    w1c_d = nc.dram_tensor("w1c", [NF, NF], bf16, kind="ExternalInput")
    w1n_d = nc.dram_tensor("w1n", [NF, NF], bf16, kind="ExternalInput")
    b1_d = nc.dram_tensor("b1", [NF, 1], f32, kind="ExternalInput")
    w2_d = nc.dram_tensor("w2", [NF, NF], bf16, kind="ExternalInput")
    b2_d = nc.dram_tensor("b2", [NF, 1], f32, kind="ExternalInput")
    out_d = nc.dram_tensor("outT", [NF, R_mlp], bf16, kind="ExternalOutput")

    with tile.TileContext(nc) as tc, ExitStack() as ctx:
        const_pool = ctx.enter_context(tc.tile_pool(name="const", bufs=1))
        agg_pool = ctx.enter_context(tc.tile_pool(name="agg", bufs=1))
        cls_pool = ctx.enter_context(tc.tile_pool(name="cls", bufs=4))
        work_pool = ctx.enter_context(tc.tile_pool(name="work", bufs=3))
        small_pool = ctx.enter_context(tc.tile_pool(name="small", bufs=4))
        mlp_pool = ctx.enter_context(tc.tile_pool(name="mlp", bufs=3))
        ctx_pool = ctx.enter_context(tc.tile_pool(name="ctxs", bufs=2))
        ctx_psum = ctx.enter_context(tc.tile_pool(name="ctxp", bufs=2, space="PSUM"))
        mlp1_psum = ctx.enter_context(tc.tile_pool(name="m1p", bufs=3, space="PSUM"))
        mlp2_psum = ctx.enter_context(tc.tile_pool(name="m2p", bufs=3, space="PSUM"))

        # prefetch first class's edge data before the const loads so phase A
        # starts immediately
        prefetched = {}
        for pi in range(min(2, len(kclasses))):
            dp, npadp, _ = kclasses[pi]
            Tp = (npadp + 127) // 128
            lgtp = cls_pool.tile([128, Tp * dp], bf16, tag="lg", name=f"lgt_pre{pi}")
            eftp = cls_pool.tile([128, Tp * dp * EF], bf16, tag="ef", name=f"eft_pre{pi}")
            nc.scalar.dma_start(lgtp[:], lg_d[pi].ap())
            nc.sync.dma_start(eftp[:], ef_d[pi].ap())
            prefetched[pi] = (lgtp, eftp)

        def load_const(name, dram, shape, dtype=f32):
            t = const_pool.tile(shape, dtype, name=name)
            nc.gpsimd.dma_start(t[:], dram.ap())
            return t

        wet4 = load_const("wet4c", wet4_d, [128, 128], bf16)
        bet4 = load_const("bet4c", bet4_d, [128, 1])
        w1c = load_const("w1cc", w1c_d, [NF, NF], bf16)
        w1n = load_const("w1nc", w1n_d, [NF, NF], bf16)
        w2 = load_const("w2c", w2_d, [NF, NF], bf16)
        b1 = load_const("b1c", b1_d, [NF, 1])
        b2 = load_const("b2c", b2_d, [NF, 1])
        zeros = const_pool.tile([128, CHUNK], f32, name="zeros")
        nc.gpsimd.memset(zeros[:], 0.0)



        aggT_blocks = []
        n_real_tiles = plan["R"] // 128
        for b in range(n_blocks):
            ab = agg_pool.tile([128, CHUNK], bf16, name=f"aggT{b}")
            used = max(0, min(16, n_real_tiles - 16 * b)) * 32
            if used < CHUNK:
                nc.gpsimd.memset(ab[0:128, used:CHUNK], 0.0)
            aggT_blocks.append(ab)

        # deg0 rows -> (block, q, col ranges) in the transposed-MLP space
        zero_ranges = {}
        if zts < plan["R"]:
            rows = np.arange(zts, plan["R"])
            cols = _mlpcol(rows)
            ch = cols // CHUNK
            cc = cols % CHUNK
            for b in range(n_blocks):
                for q in range(4):
                    sel = ch == 4 * b + q
                    if not sel.any():
                        continue
                    cs = np.sort(cc[sel])
                    runs = []
                    r0 = prev = cs[0]
                    for v in cs[1:]:
                        if v != prev + 1:
                            runs.append((q, int(r0), int(prev) + 1))
                            r0 = v
                        prev = v
                    runs.append((q, int(r0), int(prev) + 1))
                    zero_ranges.setdefault(b, []).extend(runs)

        # ---------------- Phases B+C ----------------
        def emit_block(b):
            ctx4 = ctx_psum.tile([128, CHUNK], f32, tag="ctx4", name=f"ctx4_{b}")
            nc.tensor.matmul(ctx4[:], wet4[:], aggT_blocks[b][:])
            s1 = mlp_pool.tile([128, CHUNK], f32, tag="s1", name=f"s1_{b}")
            nc.scalar.activation(s1[:], ctx4[:], mybir.ActivationFunctionType.Relu,
                                 bias=bet4[:], scale=1.0)
            s2 = mlp_pool.tile([128, CHUNK], f32, tag="s2", name=f"s2_{b}")
            nc.scalar.activation(s2[:], ctx4[:], mybir.ActivationFunctionType.Exp,
                                 bias=bet4[:], scale=1.0)
            cb = ctx_pool.tile([128, CHUNK], bf16, tag="cb", name=f"cb_{b}")
            nc.vector.scalar_tensor_tensor(cb[:], s2[:], 1.0, s1[:],
                                           mybir.AluOpType.min,
                                           mybir.AluOpType.add)
            # zero the context of deg-0 nodes (rows in [zts, R))
            for (q, c0, c1) in zero_ranges.get(b, []):
                nc.vector.memset(cb[32 * q:32 * (q + 1), c0:c1], 0.0)

            bcols = 4 * CHUNK
            nfblk = mlp_pool.tile([NF, 4 * CHUNK], bf16, tag="nfblk", name=f"nfblk{b}")
            nc.sync.dma_start(nfblk[:], nfT_d.ap()[:, b * 4 * CHUNK:(b + 1) * 4 * CHUNK])
            oblk = mlp_pool.tile([NF, 4 * CHUNK], bf16, tag="oblk", name=f"oblk{b}")
            for g in range(4):
                j = 4 * b + g
                if j >= n_chunks:
                    break
                nfb = nfblk[0:NF, g * CHUNK:(g + 1) * CHUNK]
                ps1 = mlp1_psum.tile([NF, CHUNK], f32, tag="ps1", name=f"ps1_{j}")
                if g < 3:
                    cbg = cb[32 * g:32 * (g + 1), :]
                    w1cg = w1c[32 * g:32 * (g + 1), :]
                else:
                    cb3 = mlp_pool.tile([HID, CHUNK], bf16, tag="cb3", name=f"cb3_{j}")
                    nc.vector.tensor_copy(cb3[:], cb[96:128, :])
                    cbg = cb3[:]
                    w1cg = w1c[0:HID, :]
                nc.tensor.matmul(ps1[:], w1cg, cbg, start=True, stop=False)
                nc.tensor.matmul(ps1[:], w1n[:], nfb, start=False, stop=True)
                h = mlp_pool.tile([NF, CHUNK], bf16, tag="h", name=f"h{j}")
                late = False
                if late:
                    nc.vector.scalar_tensor_tensor(h[:], ps1[:], b1[:], zeros[:],
                                                   mybir.AluOpType.add,
                                                   mybir.AluOpType.max)
                else:
                    nc.scalar.activation(h[:], ps1[:],
                                         mybir.ActivationFunctionType.Relu,
                                         bias=b1[:], scale=1.0)
                ps2 = mlp2_psum.tile([NF, CHUNK], f32, tag="ps2", name=f"ps2_{j}")
                nc.tensor.matmul(ps2[:], w2[:], h[:])
                ov = oblk[0:NF, g * CHUNK:(g + 1) * CHUNK]
                if late:
                    nc.vector.scalar_tensor_tensor(ov, ps2[:], b2[:], zeros[:],
                                                   mybir.AluOpType.add,
                                                   mybir.AluOpType.max)
                else:
                    nc.scalar.activation(ov, ps2[:],
                                         mybir.ActivationFunctionType.Relu,
                                         bias=b2[:], scale=1.0)
            nc.gpsimd.dma_start(out_d.ap()[:, b * 4 * CHUNK:b * 4 * CHUNK + bcols],
                                oblk[0:NF, 0:bcols])


        next_block = [0]

        # ---------------- Phase A ----------------
        for idx, (d, npad, off) in enumerate(kclasses):
            T = (npad + 127) // 128
            if idx in prefetched:
                lgt, eft = prefetched[idx]
            else:
                lgt = cls_pool.tile([128, T * d], bf16, tag="lg", name=f"lgt{idx}")
                eft = cls_pool.tile([128, T * d * EF], bf16, tag="ef", name=f"eft{idx}")
                nc.scalar.dma_start(lgt[:], lg_d[idx].ap())
                nc.sync.dma_start(eft[:], ef_d[idx].ap())

            x = work_pool.tile([128, T * d], bf16, tag="x", name=f"x{idx}")
            nc.scalar.activation(x[:], lgt[:], mybir.ActivationFunctionType.Exp)
            x3 = x[:].rearrange("p (t d) -> p t d", t=T)
            den = small_pool.tile([128, T], f32, tag="den", name=f"den{idx}")
            nc.vector.tensor_reduce(den[:], x3, mybir.AxisListType.X,
                                    mybir.AluOpType.add)
            rd = small_pool.tile([128, T], f32, tag="rd", name=f"rd{idx}")
            nc.vector.reciprocal(rd[:], den[:])
            xs = work_pool.tile([128, T * d], bf16, tag="xs", name=f"xs{idx}")
            xs3 = xs[:].rearrange("p (t d) -> p t d", t=T)
            rd_b = rd[:].unsqueeze(2).broadcast_to([128, T, d])
            nc.vector.tensor_tensor(xs3, x3, rd_b, mybir.AluOpType.mult)

            prod = work_pool.tile([128, T * d * EF], bf16, tag="prod", name=f"prod{idx}")
            ef4 = eft[:].rearrange("p (t f d) -> p t f d", t=T, f=EF)
            xs4 = xs3.unsqueeze(2).broadcast_to([128, T, EF, d])
            prod4 = prod[:].rearrange("p (t f d) -> p t f d", t=T, f=EF)
            nc.vector.tensor_tensor(prod4, ef4, xs4, mybir.AluOpType.mult)

            # pairwise halving (contiguous halves keep 2x bf16 mode) before
            # the 1x tensor_reduce: cuts the dominant reduce cost ~40%
            cur, w = prod, d
            lvl = 0
            while w % 2 == 0 and w > 2 and lvl < 2:
                h = w // 2
                nxt = work_pool.tile([128, T * EF * h], bf16, tag=f"ph{lvl}",
                                     name=f"ph{lvl}_{idx}")
                pa = bass.AP(cur.tensor, 0,
                             [[T * EF * w, 128], [w, T * EF], [1, h]])
                pb = bass.AP(cur.tensor, h,
                             [[T * EF * w, 128], [w, T * EF], [1, h]])
                po = bass.AP(nxt.tensor, 0,
                             [[T * EF * h, 128], [h, T * EF], [1, h]])
                nc.vector.tensor_tensor(po, pa, pb, mybir.AluOpType.add)
                cur, w = nxt, h
                lvl += 1
            a16 = work_pool.tile([128, T * EF], f32, tag="a16", name=f"a16{idx}")
            a16_3 = a16[:].rearrange("p (t f) -> p t f", t=T)
            prod_r = cur[:].rearrange("p (t f d) -> p t f d", t=T, f=EF, d=w)
            nc.vector.tensor_reduce(a16_3, prod_r, mybir.AxisListType.X,
                                    mybir.AluOpType.add)
            # padded bf16 cast: [128, T*16] -> channels 0:16 of [128, T*32]
            a16b = work_pool.tile([128, T * 32], bf16, tag="a16b", name=f"a16b{idx}")
            nc.scalar.copy(
                bass.AP(a16b.tensor, 0, [[T * 32, 128], [32, T], [EF, 2], [1, EF]]),
                bass.AP(a16.tensor, 0, [[T * EF, 128], [EF, T], [0, 2], [1, EF]]))
            # 32x32 block transpose straight into the aggT blocks
            t0 = off // 128
            t = 0
            while t < T:
                b = (t0 + t) // 16
                te = min(T, (b + 1) * 16 - t0)
                nc.vector.transpose(
                    aggT_blocks[b][0:128, 32 * (t0 + t - 16 * b):32 * (t0 + te - 16 * b)],
                    a16b[:, 32 * t:32 * te])
                t = te

            done_rows = off + npad
            if idx == len(kclasses) - 1:
                done_rows = R_mlp * 2
            while (next_block[0] < n_blocks
                   and done_rows >= min((next_block[0] + 1) * 4 * CHUNK, R_pad)):
                emit_block(next_block[0])
                next_block[0] += 1

    nc.compile()
    return nc


def kernel(**inputs):
    dst = np.asarray(inputs["dst"])
    plan = _build_plan(dst)
    in_maps = _shard_inputs(inputs, plan)
    nc = _build_kernel(plan)
    trace = bool(int(os.environ.get("GNN_PROFILE", "0")))
    if trace:
        try:
            _install_ntff_hook()
        except Exception:
            pass
    res = run_bass_kernel_spmd(nc, in_maps, core_ids=list(range(NCORES)),
                               trace=trace)
    kernel.last_results = res
    return _unshard(res.results, plan)


def _install_ntff_hook():
    """Recreate antenv.axon_hooks (absent in this image) so
    run_bass_kernel_spmd(trace=True) can NTFF-profile via libaxon_pjrt.so."""
    import contextlib, ctypes, sys, types
    if 'antenv.axon_hooks' in sys.modules:
        return
    lib = ctypes.CDLL('/opt/axon/libaxon_pjrt.so')
    lib.axon_start_nrt_profile.argtypes = [ctypes.POINTER(ctypes.c_int64), ctypes.c_size_t]
    lib.axon_start_nrt_profile.restype = ctypes.c_int64
    lib.axon_stop_nrt_profile.argtypes = [ctypes.c_char_p]
    lib.axon_stop_nrt_profile.restype = ctypes.c_int64

    @contextlib.contextmanager
    def _hook(output_dir, device_ids):
        import jax
        jax.devices()
        if device_ids:
            ids = (ctypes.c_int64 * len(device_ids))(*device_ids)
            rc = lib.axon_start_nrt_profile(ids, len(device_ids))
        else:
            rc = lib.axon_start_nrt_profile(None, 0)
        if rc != 0:
            raise RuntimeError(f"axon_start_nrt_profile rc={rc}")
        try:
            yield
        finally:
            n = lib.axon_stop_nrt_profile(str(output_dir).encode())
            print(f"ntff profile: {n} file(s) written to {output_dir}", file=sys.stderr)

    mod = types.ModuleType('antenv.axon_hooks')
    mod.get_axon_ntff_profile_hook = lambda: _hook
    mod.set_axon_ntff_profile_hook = lambda h: None
    import antenv
    antenv.axon_hooks = mod
    sys.modules['antenv.axon_hooks'] = mod

